# revision 1
# baseline (speedup 1.0000x reference)
"""RNN-T joint network kernel for 8 Trainium2 NeuronCores.

out[b,t,u,:] = W2 @ tanh(W1e @ enc[b,t] + W1d @ dec[b,u] + b1) + b2

Shapes: B=4, T=200, U=100, D=512, H=1024, O=512 (fp32 in/out).
Sharding: T split 8 ways (25 t's per core); dec + weights replicated.

All matmul inputs are bf16 (rel-err budget 2e-2; measured bf16 error
~3.8e-3; fp8 measured 3.4e-2 — over budget). bf16/fp32r both stream
at 1 cycle/row on the PE, so bf16's win is half the DMA/SBUF traffic
and no fp32r cast instructions at startup.

Input DMA facts measured on HW: the 16 DMA engines are shared by both
HWDGE rings (~370 B/ns aggregate) and each ring dispatches its queue
FIFO. Inputs are packed into wide tensors (2-11KB lines) and ordered
by need: b1 and the b=0 slice of the dec side first, W2 (first needed
~24us in) at the back of the scalar ring's FIFO.

Phase 1 is split by batch: only dech[b=0] is computed up front (16
fast 100-col matmuls); the 300-col dec matmuls for b=1..3 are emitted
as PE filler between the first chunk groups, exactly where the b0
build chain (add -> tanh) would otherwise stall the PE.

Phase 2 emission is software-pipelined by one chunk — build(i+1)
[broadcast-add + tanh], then matmuls(i), then psum copies(i) — so the
in-order DVE/ACT queues always hold ready work ahead of the
PE-dependent psum copies (avoids head-of-line blocking).  Chunks are
up to 5 t's (500-row streams; matmul moving size is ISA-limited to
512 = one PSUM bank).

Engine assignment per chunk: DVE broadcast-adds k0..6 (~110ns per
100-elem line), GpSimd only k7 (it measures ~0.8us per line), ACT does
both tanhs and 2 psum copies, DVE the other 2 (GpSimd cannot read
PSUM). Output is written bf16 (halves out-DMA); b2 and the fp32
upcast happen on the host.
"""

from contextlib import ExitStack

import ml_dtypes
import numpy as np

import concourse.bacc as bacc
import concourse.bass as bass
import concourse.mybir as mybir
import concourse.tile as tile
from concourse.bass_utils import run_bass_kernel_spmd

F32 = mybir.dt.float32
BF16 = mybir.dt.bfloat16

B, T, U, D, H, O = 4, 200, 100, 512, 1024, 512
NCORES = 8
TLOC = T // NCORES            # 25 t's per core
PAIRS = B * TLOC              # 100 (b,t) pairs per core
BU = B * U                    # 400
ROWS = PAIRS * U              # 10000 output rows per core
DK = D // 128                 # 4 contraction chunks for phase 1
HK = H // 128                 # 8 h chunks
KA = 7                        # k chunks in the A half (k=0..6, DVE)
KB = HK - KA                  # 1 k chunk in the B half (k=7, GpSimd)
CHMAX = 500                   # max rows per phase-2 chunk
U3 = 3 * U                    # 300 dec cols for b=1..3

ENC_W = DK * PAIRS            # 400
DEC0_W = DK * U               # 400: b=0 slice of decT
DEC3_W = DK * U3              # 1200: b=1..3 slice
W1H = (HK // 2) * 512         # 2048: k-major half of a W1 side

_CACHE = {}


def _chunks():
    sizes_by_b = [
        [1, 2, 3, 4, 5, 5, 5],
        [5] * 5,
        [5] * 5,
        [5, 5, 5, 5, 4, 1],
    ]
    out = []
    for b, sizes in enumerate(sizes_by_b):
        t0 = 0
        for tch in sizes:
            out.append((b, t0, tch))
            t0 += tch
        assert t0 == TLOC
    return out


def _build():
    nc = bacc.Bacc("TRN2", target_bir_lowering=False, debug=False,
                   num_devices=NCORES)
    # k-major W1 halves: col = (k % 4)*512 + dk*128 + j
    inSa = nc.dram_tensor("inSa", [128, ENC_W + W1H], BF16,
                          kind="ExternalInput")   # encT | w1e k0..3
    inSb = nc.dram_tensor("inSb", [128, W1H], BF16,
                          kind="ExternalInput")   # w1e k4..7
    inCa = nc.dram_tensor("inCa", [128, DEC0_W + W1H], BF16,
                          kind="ExternalInput")   # decT b0 | w1d k0..3
    inCb = nc.dram_tensor("inCb", [128, W1H], BF16,
                          kind="ExternalInput")   # w1d k4..7
    dec3 = nc.dram_tensor("dec3", [128, DEC3_W], BF16,
                          kind="ExternalInput")   # decT b1..3
    w2T = nc.dram_tensor("w2T", [128, HK * O], BF16, kind="ExternalInput")
    b1r = nc.dram_tensor("b1r", [128, HK], F32, kind="ExternalInput")
    out = nc.dram_tensor("out", [O, ROWS], BF16, kind="ExternalOutput")

    with tile.TileContext(nc) as tc, ExitStack() as ctx:
        consts = ctx.enter_context(tc.tile_pool(name="consts", bufs=1))
        spoolA = ctx.enter_context(tc.tile_pool(name="spoolA", bufs=3))
        spoolB = ctx.enter_context(tc.tile_pool(name="spoolB", bufs=3))
        opool = ctx.enter_context(tc.tile_pool(name="opool", bufs=8))
        psB = ctx.enter_context(tc.tile_pool(name="psB", bufs=8, space="PSUM"))

        inSa_s = consts.tile([128, ENC_W + W1H], BF16)
        inSb_s = consts.tile([128, W1H], BF16)
        inCa_s = consts.tile([128, DEC0_W + W1H], BF16)
        inCb_s = consts.tile([128, W1H], BF16)
        dec3_s = consts.tile([128, DEC3_W], BF16)
        w2_s = consts.tile([128, HK * O], BF16)
        b1_s = consts.tile([128, HK], F32)
        nc.scalar.dma_start(b1_s[:], b1r[:])
        nc.sync.dma_start(inSa_s[:], inSa[:])
        nc.scalar.dma_start(inCa_s[:], inCa[:])
        nc.sync.dma_start(inSb_s[:], inSb[:])
        nc.scalar.dma_start(inCb_s[:], inCb[:])
        nc.scalar.dma_start(dec3_s[:], dec3[:])
        nc.scalar.dma_start(w2_s[:], w2T[:])
        encT_s = inSa_s[:, :ENC_W]
        dec0_s = inCa_s[:, :DEC0_W]

        def w1e_blk(k, dk):
            if k < 4:
                return inSa_s[:, ENC_W + k * 512 + dk * 128:
                              ENC_W + k * 512 + (dk + 1) * 128]
            return inSb_s[:, (k - 4) * 512 + dk * 128:
                          (k - 4) * 512 + (dk + 1) * 128]

        def w1d_blk(k, dk):
            if k < 4:
                return inCa_s[:, DEC0_W + k * 512 + dk * 128:
                              DEC0_W + k * 512 + (dk + 1) * 128]
            return inCb_s[:, (k - 4) * 512 + dk * 128:
                          (k - 4) * 512 + (dk + 1) * 128]

        # ---- phase 1 tiles ----
        ench_t = {"A": consts.tile([128, KA * PAIRS], BF16, name="enchA"),
                  "B": consts.tile([128, KB * PAIRS], BF16, name="enchB")}
        dech0 = {"A": consts.tile([128, KA * U], BF16, name="dech0A"),
                 "B": consts.tile([128, KB * U], BF16, name="dech0B")}
        dech3 = {"A": consts.tile([128, KA * U3], BF16, name="dech3A"),
                 "B": consts.tile([128, KB * U3], BF16, name="dech3B")}

        def halfslot(k):
            return ("A", k) if k < KA else ("B", k - KA)

        def p1_enc(k):
            pe = psB.tile([128, 512], F32, tag="psB", name="pe")[:, :PAIRS]
            for dk in range(DK):
                nc.tensor.matmul(
                    pe[:], lhsT=w1e_blk(k, dk),
                    rhs=encT_s[:, dk * PAIRS:(dk + 1) * PAIRS],
                    start=(dk == 0), stop=(dk == DK - 1),
                )
            hf, kk = halfslot(k)
            nc.scalar.activation(
                ench_t[hf][:, kk * PAIRS:(kk + 1) * PAIRS], pe[:],
                mybir.ActivationFunctionType.Identity, bias=b1_s[:, k:k + 1])

        def p1_dec0(k):
            pd = psB.tile([128, 512], F32, tag="psB", name="pd")[:, :U]
            for dk in range(DK):
                nc.tensor.matmul(
                    pd[:], lhsT=w1d_blk(k, dk),
                    rhs=dec0_s[:, dk * U:(dk + 1) * U],
                    start=(dk == 0), stop=(dk == DK - 1),
                )
            hf, kk = halfslot(k)
            # all dech copies on DVE: ACT's early queue must stay clear
            # for the first tanhs (its static order can't be trusted to
            # prioritize them past queued copies)
            nc.vector.tensor_copy(dech0[hf][:, kk * U:(kk + 1) * U], pd[:])

        def p1_dec3(k):
            pd = psB.tile([128, 512], F32, tag="psB", name="pd3")[:, :U3]
            for dk in range(DK):
                nc.tensor.matmul(
                    pd[:], lhsT=w1d_blk(k, dk),
                    rhs=dec3_s[:, dk * U3:(dk + 1) * U3],
                    start=(dk == 0), stop=(dk == DK - 1),
                )
            hf, kk = halfslot(k)
            nc.vector.tensor_copy(dech3[hf][:, kk * U3:(kk + 1) * U3], pd[:])

        # ordered by DMA arrival: inSa, inSb, inCa, inCb
        for k in range(HK):
            p1_enc(k)
        for k in range(HK):
            p1_dec0(k)

        # ---- phase 2, software-pipelined by one chunk ----
        chunks = _chunks()
        s_tiles = [None] * len(chunks)
        ps_tiles = [None] * len(chunks)

        def build(i):
            b, t0c, tch = chunks[i]
            rows_c = tch * U
            s_t = {"A": spoolA.tile([128, KA * CHMAX], BF16, tag="sA",
                                    name="sA"),
                   "B": spoolB.tile([128, KB * CHMAX], BF16, tag="sB",
                                    name="sB")}
            s_tiles[i] = s_t
            for hf, nk, eng in (("B", KB, nc.gpsimd), ("A", KA, nc.vector)):
                if b == 0:
                    dech_ap = dech0[hf][:].rearrange(
                        "p (k u) -> p k u", k=nk)
                else:
                    dech_ap = dech3[hf][:].rearrange(
                        "p (k bu) -> p k bu", k=nk)[
                            :, :, (b - 1) * U:b * U]
                dech_ap = dech_ap.rearrange("p k (a u) -> p k a u", a=1)
                c0 = b * TLOC + t0c
                ench_ap = ench_t[hf][:].rearrange(
                    "p (k c) -> p k c", k=nk)[:, :, c0:c0 + tch]
                ench_ap = ench_ap.rearrange("p k (t a) -> p k t a", a=1)
                bc_d, bc_e = bass.broadcast_tensor_aps(dech_ap, ench_ap)
                outap = s_t[hf][:, :nk * rows_c].rearrange(
                    "p (k t u) -> p k t u", k=nk, t=tch)
                eng.tensor_tensor(outap, bc_d, bc_e, mybir.AluOpType.add)
                nc.scalar.activation(s_t[hf][:, :nk * rows_c],
                                     s_t[hf][:, :nk * rows_c],
                                     mybir.ActivationFunctionType.Tanh)

        def mms(i):
            b, t0c, tch = chunks[i]
            rows_c = tch * U
            s_t = s_tiles[i]
            ps = []
            for oc in range(O // 128):
                p = psB.tile([128, 512], F32, tag="psB",
                             name="p")[:, :rows_c]
                ps.append(p)
                for k in range(KA):
                    nc.tensor.matmul(
                        p[:],
                        lhsT=w2_s[:, k * O + oc * 128: k * O + (oc + 1) * 128],
                        rhs=s_t["A"][:, k * rows_c:(k + 1) * rows_c],
                        start=(k == 0), stop=False,
                    )
                k = KA
                nc.tensor.matmul(
                    p[:],
                    lhsT=w2_s[:, k * O + oc * 128: k * O + (oc + 1) * 128],
                    rhs=s_t["B"][:, :rows_c],
                    start=False, stop=True,
                )
            ps_tiles[i] = ps

        def copies(i):
            b, t0c, tch = chunks[i]
            rows_c = tch * U
            row0 = b * (TLOC * U) + t0c * U
            ps = ps_tiles[i]
            for oc in range(O // 128):
                ot = opool.tile([128, CHMAX], BF16, tag="ot",
                                name="ot")[:, :rows_c]
                # gpsimd cannot access PSUM; split copies ACT/DVE
                if oc < 2:
                    nc.scalar.activation(ot[:], ps[oc][:],
                                         mybir.ActivationFunctionType.Copy)
                else:
                    nc.vector.tensor_copy(ot[:], ps[oc][:])
                ring = nc.sync if oc % 2 == 0 else nc.scalar
                ring.dma_start(
                    out[oc * 128:(oc + 1) * 128, row0:row0 + rows_c], ot[:])

        # prologue: the dec matmuls for b=1..3 are PE filler while the
        # b0 build chain (DVE add -> ACT tanh) catches up
        build(0)
        build(1)
        mms(0)
        copies(0)
        build(2)
        for k in range(4):
            p1_dec3(k)
        mms(1)
        copies(1)
        build(3)
        for k in range(4, HK):
            p1_dec3(k)
        mms(2)
        copies(2)
        for i in range(3, len(chunks)):
            if i + 1 < len(chunks):
                build(i + 1)
            mms(i)
            copies(i)
    nc.compile()
    return nc


def _chunk128(a):
    # [n*128, w] -> [128, n*w]: partition p holds row k*128+p of chunk k
    n = a.shape[0] // 128
    return np.ascontiguousarray(
        a.reshape(n, 128, a.shape[1]).transpose(1, 0, 2).reshape(128, -1))


def _bf16(a):
    return np.ascontiguousarray(a).astype(ml_dtypes.bfloat16)


def _kmajor(w1T):
    # [128, dk-major (DK x H)] -> [128, k-major (HK x DK x 128)]
    return np.ascontiguousarray(
        w1T.reshape(128, DK, HK, 128).transpose(0, 2, 1, 3).reshape(128, -1))


def kernel(enc_state, dec_state, W1, b1, W2, b2, _trace=False):
    enc_state = np.ascontiguousarray(enc_state, dtype=np.float32)
    dec_state = np.ascontiguousarray(dec_state, dtype=np.float32)
    W1 = np.asarray(W1, dtype=np.float32)
    b1 = np.asarray(b1, dtype=np.float32)
    W2 = np.asarray(W2, dtype=np.float32)
    b2 = np.asarray(b2, dtype=np.float32)

    if "nc" not in _CACHE:
        _CACHE["nc"] = _build()
    nc = _CACHE["nc"]

    decT = _chunk128(dec_state.reshape(B * U, D).T)     # [128, DK*BU]
    dec0 = np.concatenate(
        [decT[:, dk * BU: dk * BU + U] for dk in range(DK)], axis=1)
    dec3 = np.concatenate(
        [decT[:, dk * BU + U:(dk + 1) * BU] for dk in range(DK)], axis=1)
    w1e_km = _kmajor(_chunk128(W1[:, :D].T))
    w1d_km = _kmajor(_chunk128(W1[:, D:].T))
    w2T = _bf16(_chunk128(W2.T))
    b1r = np.ascontiguousarray(b1.reshape(HK, 128).T)
    inCa = _bf16(np.concatenate([dec0, w1d_km[:, :W1H]], axis=1))
    inCb = _bf16(w1d_km[:, W1H:])
    inSb = _bf16(w1e_km[:, W1H:])
    dec3 = _bf16(dec3)

    in_maps = []
    for c in range(NCORES):
        enc_c = enc_state[:, c * TLOC:(c + 1) * TLOC, :].reshape(PAIRS, D)
        encT = _chunk128(enc_c.T)
        in_maps.append({
            "inSa": _bf16(np.concatenate([encT, w1e_km[:, :W1H]], axis=1)),
            "inSb": inSb, "inCa": inCa, "inCb": inCb, "dec3": dec3,
            "w2T": w2T, "b1r": b1r,
        })

    res = run_bass_kernel_spmd(nc, in_maps, list(range(NCORES)), trace=_trace)
    out = np.empty((B, T, U, O), dtype=np.float32)
    for c in range(NCORES):
        out[:, c * TLOC:(c + 1) * TLOC] = (
            res.results[c]["out"].astype(np.float32).T.reshape(
                B, TLOC, U, O))
    out += b2
    if _trace:
        kernel.last_results = res
    return out



# revision 2
# speedup vs baseline: 1.0539x; 1.0539x over previous
"""RNN-T joint network kernel for 8 Trainium2 NeuronCores.

out[b,t,u,:] = W2 @ tanh(W1e @ enc[b,t] + W1d @ dec[b,u] + b1) + b2

Shapes: B=4, T=200, U=100, D=512, H=1024, O=512 (fp32 in/out).

Sharding: (b, t-half) per core — core c handles b=c//2, t's
[100*(c%2), 100*(c%2)+100). Each core computes 10000 output rows;
phase 1 (ench/dech) is 6400 PE cycles (vs 16000 for t-sharding,
which replicates dech for all 4 b's on every core).

Precision: phase 1 and 6 of 8 phase-2 h-chunks are bf16; h-chunks
6,7 run as one fp8(e4m3) DoubleRow matmul per output block (2 MACs/
cycle). Measured-on-sim rel_fro ~1.75e-2 vs the 2e-2 budget (bf16
alone is 3.8e-3; full fp8 3.4e-2). W2 is pre-scaled x32 so its
values sit in e4m3's normal range; the host divides the output back.

DMA: inputs are split into k-chunk-granular pieces so the PE can
start phase 1 ~1.3us in, ordered by need on the two HWDGE rings
(sync: encT+W1e pieces then W2 oc2/3; scalar: b1, decT+W1d pieces,
then W2 oc0/1 + fp8 W2). Output is written bf16 (b2 + upcast on
host).

Phase 2 is software-pipelined by two chunks: mms(i), build(i+2),
copies(i) — the in-order DVE/ACT queues always hold ready build work
ahead of the PE-dependent psum copies. Chunk sizes [1,2,3,4] +
[5]*17 + [3,2] (rows = 100*t): small at the start so the first
builds finish while input DMA completes, small at the end so the
final build chain isn't exposed.

Engine split per chunk: DVE adds k0..4 (bf16) + k6,7 (fp8 pre-add,
bf16); GpSimd adds k5 (it measures ~6.3ns/col); ACT does the three
tanhs (the fp8 one writes e4m3 into a 512-padded pair layout for the
DoubleRow rhs, whose k-step must be 16B-aligned) and 2 of 4 psum
copies; DVE the other 2 (GpSimd cannot read PSUM).
"""

from contextlib import ExitStack

import ml_dtypes
import numpy as np

import concourse.bacc as bacc
import concourse.bass as bass
import concourse.mybir as mybir
import concourse.tile as tile
from concourse.bass_utils import run_bass_kernel_spmd

F32 = mybir.dt.float32
BF16 = mybir.dt.bfloat16
F8E4 = mybir.dt.float8e4

B, T, U, D, H, O = 4, 200, 100, 512, 1024, 512
NCORES = 8
TLOC = 100                    # t's per core (half of one b's 200)
ROWS = TLOC * U               # 10000 output rows per core
DK = D // 128                 # 4 contraction chunks for phase 1
HK = H // 128                 # 8 h chunks
NBF = 6                       # bf16 h-chunks (k0..5): 5 on DVE, 1 GpSimd
NA = 5                        # DVE bf16 chunks k0..4
CH = 500                      # max rows per phase-2 chunk
SCALE = 32.0                  # W2 pre-scale so e4m3 values are normal
W1W = HK * 512                # 4096 cols of k-major W1 per side
ENC_W = DK * TLOC             # 400
OCB = O // 128                # 4 output blocks

_CACHE = {}


def _chunks():
    sizes = [1, 2, 3, 4] + [5] * 17 + [3, 2]
    assert sum(sizes) == TLOC
    out, t0 = [], 0
    for tch in sizes:
        out.append((t0, tch))
        t0 += tch
    return out


def _build():
    nc = bacc.Bacc("TRN2", target_bir_lowering=False, debug=False,
                   num_devices=NCORES)
    inS = nc.dram_tensor("inS", [128, ENC_W + W1W], BF16,
                         kind="ExternalInput")    # encT | w1e k-major
    inC = nc.dram_tensor("inC", [128, ENC_W + W1W], BF16,
                         kind="ExternalInput")    # decT | w1d k-major
    w2b = nc.dram_tensor("w2b", [128, OCB * NBF * 128], BF16,
                         kind="ExternalInput")    # oc-major bf16 W2*32
    w2f = nc.dram_tensor("w2f", [128, OCB * 2 * 128], F8E4,
                         kind="ExternalInput")    # oc-major e4m3 W2*32, k6/7
    b1r = nc.dram_tensor("b1r", [128, HK], F32, kind="ExternalInput")
    out = nc.dram_tensor("out", [O, ROWS], BF16, kind="ExternalOutput")

    with tile.TileContext(nc) as tc, ExitStack() as ctx:
        consts = ctx.enter_context(tc.tile_pool(name="consts", bufs=1))
        spoolA = ctx.enter_context(tc.tile_pool(name="spoolA", bufs=3))
        spoolB = ctx.enter_context(tc.tile_pool(name="spoolB", bufs=3))
        ppool8 = ctx.enter_context(tc.tile_pool(name="ppool8", bufs=3))
        spool8 = ctx.enter_context(tc.tile_pool(name="spool8", bufs=3))
        opool = ctx.enter_context(tc.tile_pool(name="opool", bufs=8))
        psB = ctx.enter_context(tc.tile_pool(name="psB", bufs=8, space="PSUM"))

        # per-DMA-piece tiles so readers wait on exactly their piece
        b1_s = consts.tile([128, HK], F32)
        inS0_s = consts.tile([128, ENC_W + 512], BF16)   # encT | w1e k0
        inC0_s = consts.tile([128, ENC_W + 512], BF16)   # decT | w1d k0
        inSk_s = [consts.tile([128, 512], BF16, name=f"inSk{k}")
                  for k in range(1, HK)]
        inCk_s = [consts.tile([128, 512], BF16, name=f"inCk{k}")
                  for k in range(1, HK)]
        w2b_s = [consts.tile([128, NBF * 128], BF16, name=f"w2b{oc}")
                 for oc in range(OCB)]
        w2f_s = consts.tile([128, OCB * 2 * 128], F8E4)

        nc.scalar.dma_start(b1_s[:], b1r[:])
        nc.sync.dma_start(inS0_s[:], inS[:, :ENC_W + 512])
        nc.scalar.dma_start(inC0_s[:], inC[:, :ENC_W + 512])
        for k in range(1, HK):
            sl = slice(ENC_W + k * 512, ENC_W + (k + 1) * 512)
            nc.sync.dma_start(inSk_s[k - 1][:], inS[:, sl])
            nc.scalar.dma_start(inCk_s[k - 1][:], inC[:, sl])
        wbw = NBF * 128
        nc.scalar.dma_start(w2b_s[0][:], w2b[:, :wbw])
        nc.scalar.dma_start(w2f_s[:], w2f[:])
        nc.scalar.dma_start(w2b_s[1][:], w2b[:, wbw:2 * wbw])
        nc.sync.dma_start(w2b_s[2][:], w2b[:, 2 * wbw:3 * wbw])
        nc.sync.dma_start(w2b_s[3][:], w2b[:, 3 * wbw:])

        def w1_blk(side, k, dk):
            if k == 0:
                t = inS0_s if side == "e" else inC0_s
                return t[:, ENC_W + dk * 128:ENC_W + (dk + 1) * 128]
            t = (inSk_s if side == "e" else inCk_s)[k - 1]
            return t[:, dk * 128:(dk + 1) * 128]

        encT = inS0_s[:, :ENC_W]
        decT = inC0_s[:, :ENC_W]

        # ---- phase 1: ench[p, k*100+t], dech[p, k*100+u], both bf16 ----
        ench_s = consts.tile([128, HK * TLOC], BF16)
        dech_s = consts.tile([128, HK * U], BF16)
        for k in range(HK):
            pe = psB.tile([128, 512], F32, tag="psB", name="pe")[:, :TLOC]
            for dk in range(DK):
                nc.tensor.matmul(
                    pe[:], lhsT=w1_blk("e", k, dk),
                    rhs=encT[:, dk * TLOC:(dk + 1) * TLOC],
                    start=(dk == 0), stop=(dk == DK - 1))
            nc.scalar.activation(
                ench_s[:, k * TLOC:(k + 1) * TLOC], pe[:],
                mybir.ActivationFunctionType.Identity, bias=b1_s[:, k:k + 1])
            pd = psB.tile([128, 512], F32, tag="psB", name="pd")[:, :U]
            for dk in range(DK):
                nc.tensor.matmul(
                    pd[:], lhsT=w1_blk("d", k, dk),
                    rhs=decT[:, dk * U:(dk + 1) * U],
                    start=(dk == 0), stop=(dk == DK - 1))
            nc.vector.tensor_copy(dech_s[:, k * U:(k + 1) * U], pd[:])

        # ---- phase 2 ----
        chunks = _chunks()
        n_ch = len(chunks)
        sA_t = [None] * n_ch
        sB_t = [None] * n_ch
        s8_t = [None] * n_ch
        ps_t = [None] * n_ch

        def bcast_add(eng, outap, k0, nk, t0c, tch, rows_c):
            dech_ap = dech_s[:, k0 * U:(k0 + nk) * U].rearrange(
                "p (k u) -> p k u", k=nk).rearrange(
                "p k (a u) -> p k a u", a=1)
            ench_ap = ench_s[:, k0 * TLOC:(k0 + nk) * TLOC].rearrange(
                "p (k t) -> p k t", k=nk)[:, :, t0c:t0c + tch].rearrange(
                "p k (t a) -> p k t a", a=1)
            bc_d, bc_e = bass.broadcast_tensor_aps(dech_ap, ench_ap)
            eng.tensor_tensor(outap, bc_d, bc_e, mybir.AluOpType.add)

        def build(i):
            t0c, tch = chunks[i]
            rows_c = tch * U
            sA = spoolA.tile([128, NA * CH], BF16, tag="sA", name="sA")
            sB = spoolB.tile([128, CH], BF16, tag="sB", name="sB")
            p8 = ppool8.tile([128, 2 * CH], BF16, tag="p8", name="p8")
            s8 = spool8.tile([128, 2 * 512], F8E4, tag="s8", name="s8")
            sA_t[i], sB_t[i], s8_t[i] = sA, sB, s8
            TANH = mybir.ActivationFunctionType.Tanh
            # A: k0..4 on DVE, then tanh in place
            bcast_add(nc.vector,
                      sA[:, :NA * rows_c].rearrange(
                          "p (k t u) -> p k t u", k=NA, t=tch),
                      0, NA, t0c, tch, rows_c)
            nc.scalar.activation(sA[:, :NA * rows_c], sA[:, :NA * rows_c],
                                 TANH)
            # B: k5 on GpSimd, tanh in place
            bcast_add(nc.gpsimd,
                      sB[:, :rows_c].rearrange(
                          "p (k t u) -> p k t u", k=1, t=tch),
                      NA, 1, t0c, tch, rows_c)
            nc.scalar.activation(sB[:, :rows_c], sB[:, :rows_c], TANH)
            # F8: k6,7 pre-add on DVE (bf16), tanh -> e4m3 pair layout
            bcast_add(nc.vector,
                      p8[:, :2 * rows_c].rearrange(
                          "p (k t u) -> p k t u", k=2, t=tch),
                      NBF, 2, t0c, tch, rows_c)
            s8_ap = s8[:].rearrange("p (j c) -> p j c", j=2)[:, :, :rows_c]
            p8_ap = p8[:, :2 * rows_c].rearrange("p (j c) -> p j c", j=2)
            nc.scalar.activation(s8_ap, p8_ap, TANH)

        def mms(i):
            t0c, tch = chunks[i]
            rows_c = tch * U
            sA, sB, s8 = sA_t[i], sB_t[i], s8_t[i]
            ps = []
            for oc in range(OCB):
                p = psB.tile([128, 512], F32, tag="psB",
                             name="p")[:, :rows_c]
                ps.append(p)
                for k in range(NA):
                    nc.tensor.matmul(
                        p[:], lhsT=w2b_s[oc][:, k * 128:(k + 1) * 128],
                        rhs=sA[:, k * rows_c:(k + 1) * rows_c],
                        start=(k == 0), stop=False)
                nc.tensor.matmul(
                    p[:], lhsT=w2b_s[oc][:, NA * 128:(NA + 1) * 128],
                    rhs=sB[:, :rows_c], start=False, stop=False)
                nc.tensor.matmul(
                    p[:],
                    lhsT=w2f_s[:, oc * 256:(oc + 1) * 256].rearrange(
                        "p (j f) -> p j f", j=2),
                    rhs=s8[:].rearrange("p (j c) -> p j c", j=2)[:, :, :rows_c],
                    start=False, stop=True,
                    perf_mode=mybir.MatmulPerfMode.DoubleRow)
            ps_t[i] = ps

        def copies(i):
            t0c, tch = chunks[i]
            rows_c = tch * U
            row0 = t0c * U
            ps = ps_t[i]
            for oc in range(OCB):
                ot = opool.tile([128, CH], BF16, tag="ot",
                                name="ot")[:, :rows_c]
                if oc < 2:
                    nc.scalar.activation(ot[:], ps[oc][:],
                                         mybir.ActivationFunctionType.Copy)
                else:
                    nc.vector.tensor_copy(ot[:], ps[oc][:])
                ring = nc.sync if oc % 2 == 0 else nc.scalar
                ring.dma_start(
                    out[oc * 128:(oc + 1) * 128, row0:row0 + rows_c], ot[:])

        build(0)
        build(1)
        for i in range(n_ch):
            mms(i)
            if i + 2 < n_ch:
                build(i + 2)
            copies(i)
    nc.compile()
    return nc


def _chunk128(a):
    # [n*128, w] -> [128, n*w]: partition p holds row k*128+p of chunk k
    n = a.shape[0] // 128
    return np.ascontiguousarray(
        a.reshape(n, 128, a.shape[1]).transpose(1, 0, 2).reshape(128, -1))


def _bf16(a):
    return np.ascontiguousarray(a).astype(ml_dtypes.bfloat16)


def _kmajor(w1T):
    # [128, dk-major (DK x H)] -> [128, k-major (HK x DK x 128)]
    return np.ascontiguousarray(
        w1T.reshape(128, DK, HK, 128).transpose(0, 2, 1, 3).reshape(128, -1))


def kernel(enc_state, dec_state, W1, b1, W2, b2, _trace=False):
    enc_state = np.ascontiguousarray(enc_state, dtype=np.float32)
    dec_state = np.ascontiguousarray(dec_state, dtype=np.float32)
    W1 = np.asarray(W1, dtype=np.float32)
    b1 = np.asarray(b1, dtype=np.float32)
    W2 = np.asarray(W2, dtype=np.float32)
    b2 = np.asarray(b2, dtype=np.float32)

    if "nc" not in _CACHE:
        _CACHE["nc"] = _build()
    nc = _CACHE["nc"]

    w1e_km = _bf16(_kmajor(_chunk128(W1[:, :D].T)))
    w1d_km = _bf16(_kmajor(_chunk128(W1[:, D:].T)))
    b1r = np.ascontiguousarray(b1.reshape(HK, 128).T)

    # W2*32, chunked [128, hk, o]: element [p, hk, o] = 32*W2[o, hk*128+p]
    w2c = _chunk128((W2.T * SCALE).astype(np.float32)).reshape(128, HK, O)
    # bf16 part, oc-major: [p, oc, k, j] for k in 0..NBF-1
    w2b = _bf16(np.ascontiguousarray(
        w2c[:, :NBF, :].reshape(128, NBF, OCB, 128).transpose(0, 2, 1, 3)
        .reshape(128, -1)))
    # fp8 part: [p, oc, jj, j] for k = NBF+jj
    w2f = np.ascontiguousarray(
        w2c[:, NBF:, :].reshape(128, 2, OCB, 128).transpose(0, 2, 1, 3)
        .reshape(128, -1)).astype(ml_dtypes.float8_e4m3)

    decT = {}
    for b in range(B):
        decT[b] = _bf16(_chunk128(dec_state[b].T))          # [128, DK*U]

    in_maps = []
    for c in range(NCORES):
        b, th = c // 2, c % 2
        enc_c = enc_state[b, th * TLOC:(th + 1) * TLOC]     # [100, 512]
        encT = _bf16(_chunk128(enc_c.T))                    # [128, DK*100]
        in_maps.append({
            "inS": np.concatenate([encT, w1e_km], axis=1),
            "inC": np.concatenate([decT[b], w1d_km], axis=1),
            "w2b": w2b, "w2f": w2f, "b1r": b1r,
        })

    res = run_bass_kernel_spmd(nc, in_maps, list(range(NCORES)), trace=_trace)
    out = np.empty((B, T, U, O), dtype=np.float32)
    for c in range(NCORES):
        b, th = c // 2, c % 2
        out[b, th * TLOC:(th + 1) * TLOC] = (
            res.results[c]["out"].astype(np.float32).T.reshape(
                TLOC, U, O)) / SCALE
    out += b2
    if _trace:
        kernel.last_results = res
    return out


# revision 9
# speedup vs baseline: 1.1030x; 1.0466x over previous
"""RNN-T joint network kernel for 8 Trainium2 NeuronCores.

out[b,t,u,:] = W2 @ tanh(W1e @ enc[b,t] + W1d @ dec[b,u] + b1) + b2

Shapes: B=4, T=200, U=100, D=512, H=1024, O=512 (fp32 in/out).

Sharding: (b, t-half) per core — core c handles b=c//2, t's
[100*(c%2), 100*(c%2)+100). Each core computes 10000 output rows;
phase 1 (ench/dech) is 6400 PE cycles (vs 16000 for t-sharding,
which replicates dech for all 4 b's on every core).

Precision: phase 1 and 6 of 8 phase-2 h-chunks are bf16; h-chunks
6,7 run as one fp8(e4m3) DoubleRow matmul per output block (2 MACs/
cycle; measured: the DR matmul costs the same issue time as ONE bf16
matmul, halving those chunks' PE time). Measured rel_fro 1.75e-2 vs
the 2e-2 budget (bf16 alone 3.8e-3; full fp8 3.4e-2). W2 is
pre-scaled x32 so its e4m3 values sit in the normal range; the host
divides the output back.

DMA facts measured on HW: a dma_start costs ~600-790ns of ISSUING
ENGINE time (plus ring-credit waits), and the two HWDGE rings are
the Sync and ACT engines. v2 put 11 input DMAs + half the output
DMAs on ACT, which saturated it (89% busy) and delayed phase-2 start
to 21.5us. Now: ACT's ring carries only 6 input pieces (interleaved
with phase-1 emission so identities aren't head-of-line blocked);
ALL output DMA rides the Sync ring (PSUM cannot be a DMA source, so
the psum copies stay on engines: 1 ACT + 3 DVE).

Inputs are split into 2-k-chunk pieces so the PE can start phase 1
as soon as piece 0 lands; 6 dummy 400-col matmuls after phase 1 keep
the PE HAM activity monitor busy so phase 2 starts at the warm 2.4
GHz clock instead of 1.2.

Phase 2 is software-pipelined by two chunks (three near the tail):
mms(i), build(i+2..), copies(i) — the in-order DVE/ACT queues always
hold ready build work ahead of the PE-dependent psum copies. Chunk
sizes [1,2,3,4] + [5]*16 + [4,3,2,1] (rows = 100*t): small at the
start so the first builds finish while input DMA completes, tapered
at the end so the final build chain isn't exposed.

Engine split per 500-row chunk (PE 5.94us): DVE adds k0..4 + the
k6,7 fp8 pre-add + 3 psum copies (~5.3us); GpSimd adds k5 (~3.1us);
ACT tanhs A/B/fp8 + 1 psum copy (~4.8us); Sync ring 4 output DMAs.
"""

from contextlib import ExitStack

import ml_dtypes
import numpy as np

import concourse.bacc as bacc
import concourse.bass as bass
import concourse.mybir as mybir
import concourse.tile as tile
from concourse.bass_utils import run_bass_kernel_spmd

F32 = mybir.dt.float32
BF16 = mybir.dt.bfloat16
F8E4 = mybir.dt.float8e4

B, T, U, D, H, O = 4, 200, 100, 512, 1024, 512
NCORES = 8
TLOC = 100                    # t's per core (half of one b's 200)
ROWS = TLOC * U               # 10000 output rows per core
DK = D // 128                 # 4 contraction chunks for phase 1
HK = H // 128                 # 8 h chunks
NBF = 6                       # bf16 h-chunks (k0..5): 5 on DVE, 1 GpSimd
NA = 5                        # DVE bf16 chunks k0..4
CH = 500                      # max rows per phase-2 chunk
SCALE = 32.0                  # W2 pre-scale so e4m3 values are normal
ENC_W = DK * TLOC             # 400
OCB = O // 128                # 4 output blocks

_CACHE = {}


def _chunks():
    sizes = [1, 2, 3, 4] + [5] * 16 + [4, 3, 2, 1]
    assert sum(sizes) == TLOC
    out, t0 = [], 0
    for tch in sizes:
        out.append((t0, tch))
        t0 += tch
    return out


def _build():
    nc = bacc.Bacc("TRN2", target_bir_lowering=False, debug=False,
                   num_devices=NCORES)
    inS = nc.dram_tensor("inS", [128, ENC_W + HK * 512], BF16,
                         kind="ExternalInput")    # encT | w1e k-major
    inC = nc.dram_tensor("inC", [128, ENC_W + HK * 512], BF16,
                         kind="ExternalInput")    # decT | w1d k-major
    w2b = nc.dram_tensor("w2b", [128, OCB * NBF * 128], BF16,
                         kind="ExternalInput")    # oc-major bf16 W2*32
    w2f = nc.dram_tensor("w2f", [128, OCB * 2 * 128], F8E4,
                         kind="ExternalInput")    # oc-major e4m3 W2*32, k6/7
    b1r = nc.dram_tensor("b1r", [128, HK], F32, kind="ExternalInput")
    out = nc.dram_tensor("out", [O, ROWS], BF16, kind="ExternalOutput")

    with tile.TileContext(nc) as tc, ExitStack() as ctx:
        consts = ctx.enter_context(tc.tile_pool(name="consts", bufs=1))
        spoolA = ctx.enter_context(tc.tile_pool(name="spoolA", bufs=5))
        spoolB = ctx.enter_context(tc.tile_pool(name="spoolB", bufs=5))
        ppool8 = ctx.enter_context(tc.tile_pool(name="ppool8", bufs=5))
        spool8 = ctx.enter_context(tc.tile_pool(name="spool8", bufs=5))
        opool = ctx.enter_context(tc.tile_pool(name="opool", bufs=8))
        psB = ctx.enter_context(tc.tile_pool(name="psB", bufs=8, space="PSUM"))

        b1_s = consts.tile([128, HK], F32)
        inS_s = [consts.tile([128, 1424 if g == 0 else 1024], BF16,
                             name=f"inS{g}") for g in range(4)]
        inC_s = [consts.tile([128, 1424 if g == 0 else 1024], BF16,
                             name=f"inC{g}") for g in range(4)]
        w2b_s = [consts.tile([128, NBF * 128], BF16, name=f"w2b{oc}")
                 for oc in range(OCB)]
        w2f_s = consts.tile([128, OCB * 2 * 128], F8E4)
        ench_s = consts.tile([128, HK * TLOC], BF16)
        dech_s = consts.tile([128, HK * U], BF16)

        def w1_blk(side, k, dk):
            t = (inS_s if side == "e" else inC_s)[k // 2]
            off = (0 if k // 2 else ENC_W) + (k % 2) * 512
            return t[:, off + dk * 128:off + (dk + 1) * 128]

        encT = inS_s[0][:, :ENC_W]
        decT = inC_s[0][:, :ENC_W]

        def p1(side, k):
            n = TLOC if side == "e" else U
            src = encT if side == "e" else decT
            p = psB.tile([128, 512], F32, tag="psB", name="p1")[:, :n]
            for dk in range(DK):
                nc.tensor.matmul(
                    p[:], lhsT=w1_blk(side, k, dk),
                    rhs=src[:, dk * n:(dk + 1) * n],
                    start=(dk == 0), stop=(dk == DK - 1))
            if side == "e":
                nc.scalar.activation(
                    ench_s[:, k * TLOC:(k + 1) * TLOC], p[:],
                    mybir.ActivationFunctionType.Identity,
                    bias=b1_s[:, k:k + 1])
            else:
                nc.vector.tensor_copy(dech_s[:, k * U:(k + 1) * U], p[:])

        # ---- input DMA, interleaved with phase-1 emission ----
        nc.sync.dma_start(b1_s[:], b1r[:])
        W = ENC_W + 1024
        nc.sync.dma_start(inS_s[0][:], inS[:, :W])
        nc.scalar.dma_start(inC_s[0][:], inC[:, :W])
        nc.sync.dma_start(inS_s[1][:], inS[:, W:W + 1024])
        nc.scalar.dma_start(inC_s[1][:], inC[:, W:W + 1024])
        p1("e", 0); p1("d", 0); p1("e", 1); p1("d", 1)
        nc.sync.dma_start(inS_s[2][:], inS[:, W + 1024:W + 2048])
        nc.scalar.dma_start(inC_s[2][:], inC[:, W + 1024:W + 2048])
        p1("e", 2); p1("d", 2); p1("e", 3); p1("d", 3)
        nc.sync.dma_start(inS_s[3][:], inS[:, W + 2048:W + 3072])
        nc.scalar.dma_start(inC_s[3][:], inC[:, W + 2048:W + 3072])
        p1("e", 4); p1("d", 4); p1("e", 5); p1("d", 5)
        wbw = NBF * 128
        nc.sync.dma_start(w2b_s[0][:], w2b[:, :wbw])
        nc.scalar.dma_start(w2b_s[1][:], w2b[:, wbw:2 * wbw])
        p1("e", 6); p1("d", 6); p1("e", 7); p1("d", 7)
        nc.sync.dma_start(w2b_s[2][:], w2b[:, 2 * wbw:3 * wbw])
        nc.scalar.dma_start(w2f_s[:], w2f[:])
        nc.sync.dma_start(w2b_s[3][:], w2b[:, 3 * wbw:])

        # keep the PE HAM window busy so phase 2 starts at 2.4 GHz
        pdum = psB.tile([128, 512], F32, tag="psB", name="pdum")[:, :ENC_W]
        for _ in range(6):
            nc.tensor.matmul(pdum[:], lhsT=w1_blk("e", 0, 0),
                             rhs=encT[:, :ENC_W], start=True, stop=True)

        # ---- phase 2 ----
        chunks = _chunks()
        n_ch = len(chunks)
        sA_t = [None] * n_ch
        sB_t = [None] * n_ch
        s8_t = [None] * n_ch
        ps_t = [None] * n_ch

        def bcast_add(eng, outap, k0, nk, t0c, tch):
            dech_ap = dech_s[:, k0 * U:(k0 + nk) * U].rearrange(
                "p (k u) -> p k u", k=nk).rearrange(
                "p k (a u) -> p k a u", a=1)
            ench_ap = ench_s[:, k0 * TLOC:(k0 + nk) * TLOC].rearrange(
                "p (k t) -> p k t", k=nk)[:, :, t0c:t0c + tch].rearrange(
                "p k (t a) -> p k t a", a=1)
            bc_d, bc_e = bass.broadcast_tensor_aps(dech_ap, ench_ap)
            eng.tensor_tensor(outap, bc_d, bc_e, mybir.AluOpType.add)

        def build(i):
            t0c, tch = chunks[i]
            rows_c = tch * U
            sA = spoolA.tile([128, NA * CH], BF16, tag="sA", name="sA")
            sB = spoolB.tile([128, CH], BF16, tag="sB", name="sB")
            p8 = ppool8.tile([128, 2 * CH], BF16, tag="p8", name="p8")
            s8 = spool8.tile([128, 2 * 512], F8E4, tag="s8", name="s8")
            sA_t[i], sB_t[i], s8_t[i] = sA, sB, s8
            TANH = mybir.ActivationFunctionType.Tanh
            bcast_add(nc.vector,
                      sA[:, :NA * rows_c].rearrange(
                          "p (k t u) -> p k t u", k=NA, t=tch),
                      0, NA, t0c, tch)
            nc.scalar.activation(sA[:, :NA * rows_c], sA[:, :NA * rows_c],
                                 TANH)
            bcast_add(nc.gpsimd,
                      sB[:, :rows_c].rearrange(
                          "p (k t u) -> p k t u", k=1, t=tch),
                      NA, 1, t0c, tch)
            nc.scalar.activation(sB[:, :rows_c], sB[:, :rows_c], TANH)
            bcast_add(nc.vector,
                      p8[:, :2 * rows_c].rearrange(
                          "p (k t u) -> p k t u", k=2, t=tch),
                      NBF, 2, t0c, tch)
            s8_ap = s8[:].rearrange("p (j c) -> p j c", j=2)[:, :, :rows_c]
            p8_ap = p8[:, :2 * rows_c].rearrange("p (j c) -> p j c", j=2)
            nc.scalar.activation(s8_ap, p8_ap, TANH)

        def mms(i):
            t0c, tch = chunks[i]
            rows_c = tch * U
            sA, sB, s8 = sA_t[i], sB_t[i], s8_t[i]
            ps = []
            for oc in range(OCB):
                p = psB.tile([128, 512], F32, tag="psB",
                             name="p")[:, :rows_c]
                ps.append(p)
                for k in range(NA):
                    nc.tensor.matmul(
                        p[:], lhsT=w2b_s[oc][:, k * 128:(k + 1) * 128],
                        rhs=sA[:, k * rows_c:(k + 1) * rows_c],
                        start=(k == 0), stop=False)
                nc.tensor.matmul(
                    p[:], lhsT=w2b_s[oc][:, NA * 128:(NA + 1) * 128],
                    rhs=sB[:, :rows_c], start=False, stop=False)
                nc.tensor.matmul(
                    p[:],
                    lhsT=w2f_s[:, oc * 256:(oc + 1) * 256].rearrange(
                        "p (j f) -> p j f", j=2),
                    rhs=s8[:].rearrange("p (j c) -> p j c", j=2)[:, :, :rows_c],
                    start=False, stop=True,
                    perf_mode=mybir.MatmulPerfMode.DoubleRow)
            ps_t[i] = ps

        def copies(i):
            t0c, tch = chunks[i]
            rows_c = tch * U
            row0 = t0c * U
            ps = ps_t[i]
            for oc in range(OCB):
                ot = opool.tile([128, CH], BF16, tag="ot",
                                name="ot")[:, :rows_c]
                if oc == 0:
                    nc.scalar.activation(ot[:], ps[oc][:],
                                         mybir.ActivationFunctionType.Copy)
                else:
                    nc.vector.tensor_copy(ot[:], ps[oc][:])
                nc.sync.dma_start(
                    out[oc * 128:(oc + 1) * 128, row0:row0 + rows_c], ot[:])

        build(0)
        build(1)
        built = 2
        for i in range(n_ch):
            mms(i)
            depth = 2 if i < n_ch - 8 else 3
            while built < min(i + depth, n_ch):
                build(built)
                built += 1
            copies(i)
    nc.compile()
    return nc


def _chunk128(a):
    # [n*128, w] -> [128, n*w]: partition p holds row k*128+p of chunk k
    n = a.shape[0] // 128
    return np.ascontiguousarray(
        a.reshape(n, 128, a.shape[1]).transpose(1, 0, 2).reshape(128, -1))


def _bf16(a):
    return np.ascontiguousarray(a).astype(ml_dtypes.bfloat16)


def _kmajor(w1T):
    # [128, dk-major (DK x H)] -> [128, k-major (HK x DK x 128)]
    return np.ascontiguousarray(
        w1T.reshape(128, DK, HK, 128).transpose(0, 2, 1, 3).reshape(128, -1))


def kernel(enc_state, dec_state, W1, b1, W2, b2, _trace=False):
    enc_state = np.ascontiguousarray(enc_state, dtype=np.float32)
    dec_state = np.ascontiguousarray(dec_state, dtype=np.float32)
    W1 = np.asarray(W1, dtype=np.float32)
    b1 = np.asarray(b1, dtype=np.float32)
    W2 = np.asarray(W2, dtype=np.float32)
    b2 = np.asarray(b2, dtype=np.float32)

    if "nc" not in _CACHE:
        _CACHE["nc"] = _build()
    nc = _CACHE["nc"]

    w1e_km = _bf16(_kmajor(_chunk128(W1[:, :D].T)))
    w1d_km = _bf16(_kmajor(_chunk128(W1[:, D:].T)))
    b1r = np.ascontiguousarray(b1.reshape(HK, 128).T)

    # W2*32, chunked [128, hk, o]: element [p, hk, o] = 32*W2[o, hk*128+p]
    w2c = _chunk128((W2.T * SCALE).astype(np.float32)).reshape(128, HK, O)
    w2b = _bf16(np.ascontiguousarray(
        w2c[:, :NBF, :].reshape(128, NBF, OCB, 128).transpose(0, 2, 1, 3)
        .reshape(128, -1)))
    w2f = np.ascontiguousarray(
        w2c[:, NBF:, :].reshape(128, 2, OCB, 128).transpose(0, 2, 1, 3)
        .reshape(128, -1)).astype(ml_dtypes.float8_e4m3)

    decT = {}
    for b in range(B):
        decT[b] = _bf16(_chunk128(dec_state[b].T))          # [128, DK*U]

    in_maps = []
    for c in range(NCORES):
        b, th = c // 2, c % 2
        enc_c = enc_state[b, th * TLOC:(th + 1) * TLOC]     # [100, 512]
        encT = _bf16(_chunk128(enc_c.T))                    # [128, DK*100]
        in_maps.append({
            "inS": np.concatenate([encT, w1e_km], axis=1),
            "inC": np.concatenate([decT[b], w1d_km], axis=1),
            "w2b": w2b, "w2f": w2f, "b1r": b1r,
        })

    res = run_bass_kernel_spmd(nc, in_maps, list(range(NCORES)), trace=_trace)
    out = np.empty((B, T, U, O), dtype=np.float32)
    for c in range(NCORES):
        b, th = c // 2, c % 2
        full = res.results[c]["out"].astype(np.float32) / SCALE
        out[b, th * TLOC:(th + 1) * TLOC] = full.T.reshape(TLOC, U, O)
    out += b2
    if _trace:
        kernel.last_results = res
    return out
